# revision 48
# baseline (speedup 1.0000x reference)
"""BoxFilter 9x9 mean, TRN2 x8 — v6: scan + matmul-differencing.

Per 128-row block (9 overlapping blocks per 1024x1024 image, 6 images/core):
  - casting DMA loads x f32->f16 into a persistent 9-chunk SBUF tile
    (chunks padded with 5 left + 4 right zero cols, zeroed once at startup);
    interior chunks are fetched with one multi-window DMA (overlapping DRAM
    windows, partition-first SBUF AP) to amortize SWDGE descriptor-gen cost
  - one DVE tensor_tensor_scan per chunk produces the horizontal prefix sum
    c (f16 out, fp32 internal state): h[j] = c[j+9] - c[j] is the 9-wide
    horizontally clamped window sum
  - 4 matmuls per block compute the vertical 9-band sum of h directly from c:
    ps = (+W)@c[:, 9:] + (-W)@c[:, :1024], with the 1/(9*vcount) row scale
    folded into +/-W (f16 weights, 3 block kinds x 2 signs)
  - one 1024-wide scalar-engine copy drains the 2-bank PSUM tile to SBUF int8
    (the tolerance is relative to max|out| ~ 0.97, so a uniform int8 grid at
    OUT_SCALE=120 costs only ~4e-3 absolute error); host dequantizes by
    1/OUT_SCALE and applies the edge-column 9/hcount fix
  - chunk-pair batched DMAs write the int8 output back (halves output DMA
    traffic vs f16)

Engine busy per core (TimelineSim): DVE ~63us (scans; 1x rate, no 2x mode
for TensorScalarPtr, and the Pool engine cannot run it), DMA ~55us (f16 in
13.4MB + int8 out 6.3MB at 360GB/s), Act ~55us (drains), PE ~46us.
fp8 input would cut DMA by another 18.5us but its quantization alone
measures 2.9e-2 max-rel error (> 2e-2 budget), so the input stays f16.
"""

import threading

import numpy as np

NCORES = 8
B, C, H, W = 16, 3, 1024, 1024
IMGS = B * C
IPC = IMGS // NCORES
R = 4
OB = 120
NBLK = 9
P = 1040  # SBUF chunk pitch (elements)
CW = 1033  # scan width: 5 left zeros + 1024 + 4 right zeros
GW = 2 * P + CW  # 3-chunk group scan width (chunk gaps are all zeros)

# (out_start, out_rows, in_start, in_rows, weight_kind)
BLOCKS = [(0, OB, 0, 128, 0)]
BLOCKS += [(OB * t, OB, OB * t - R, 128, 1) for t in range(1, 8)]
BLOCKS += [(960, 64, 956, 68, 2)]

INPUT_FP8 = False  # False -> f16 input path
OUT_SCALE = 120.0  # output quantization scale for int8 DRAM writes
POOL_SCANS = 0  # gpsimd cannot run TensorScalarPtr (HW ISA check) — keep 0
DVE_DRAINS = 0  # drains per image on DVE instead of Act


def _weights():
    k = np.arange(128)[:, None].astype(np.int64)
    m = np.arange(128)[None, :].astype(np.int64)
    vc0 = np.minimum(m + R, 1023) - np.maximum(m - R, 0) + 1
    w0 = ((np.maximum(m - R, 0) <= k) & (k <= m + R) & (m < OB)) / (9.0 * vc0)
    wi = ((m <= k) & (k <= m + 2 * R) & (m < OB)) / 81.0
    vc8 = np.maximum(np.minimum(964 + m, 1023) - (956 + m) + 1, 1)
    w8 = ((m <= k) & (k <= np.minimum(m + 2 * R, 67)) & (m < 64)) / (9.0 * vc8)
    wts = np.stack([w0, wi, w8]) * OUT_SCALE
    wts6 = np.concatenate([wts, -wts]).astype(np.float16)  # [6,128,128]
    return np.ascontiguousarray(wts6.transpose(1, 0, 2).reshape(128, 6 * 128))


def _build(reps: int = 1):
    import concourse.bacc as bacc
    import concourse.mybir as mybir
    import concourse.tile as tile
    from concourse.ap import AP

    f32 = mybir.dt.float32
    f16 = mybir.dt.float16
    fp8 = mybir.dt.float8e4
    xdt = fp8 if INPUT_FP8 else f16
    add = mybir.AluOpType.add

    nc = bacc.Bacc("TRN2", target_bir_lowering=False, debug=False, num_devices=NCORES)
    x_d = nc.declare_dram_parameter("x", [IPC, H, W], f32, isOutput=False)
    wts_d = nc.declare_dram_parameter("wts", [128, 6 * 128], f16, isOutput=False)
    i8 = mybir.dt.int8
    o_d = nc.declare_dram_parameter("out", [IPC, H, W], i8, isOutput=True)

    NXB = 3
    NOB = 4
    with tile.TileContext(nc) as tc:
        with (
            tc.tile_pool(name="consts", bufs=1) as cpool,
            tc.tile_pool(name="xb", bufs=NXB) as xb_pool,
            tc.tile_pool(name="cs", bufs=5) as cs_pool,
            tc.tile_pool(name="cg", bufs=3) as c_pool,
            tc.tile_pool(name="ob", bufs=NOB) as ob_pool,
            tc.tile_pool(name="ps", bufs=4, space="PSUM") as ps_pool,
        ):
            w_sb = cpool.tile([128, 6 * 128], f16)
            nc.sync.dma_start(out=w_sb[:, 0:384], in_=wts_d[:, 0:384])
            nc.sync.dma_start(out=w_sb[:, 384:768], in_=wts_d[:, 384:768])
            zeros = cpool.tile([128, CW], f16)

            # persistent multi-buffered input / output tiles
            xbs = [xb_pool.tile([128, NBLK * P], xdt, tag="xb", name=f"xb{i}")
                   for i in range(NXB)]
            obs = [ob_pool.tile([128, NBLK * 1024], i8, tag="ob", name=f"ob{i}")
                   for i in range(NOB)]
            def pads(xb, eng):
                nat = xb[0:128, 0 : NBLK * P]
                pstride = nat.ap[0][0]
                padl = AP(nat.tensor, nat.offset, [[pstride, 128], [P, NBLK], [1, 5]])
                padr = AP(nat.tensor, nat.offset + 1029,
                          [[pstride, 128], [P, NBLK], [1, 11]])
                eng.memset(padl, 0.0)
                eng.memset(padr, 0.0)

            pads(xbs[0], nc.vector)
            nc.vector.memset(zeros[:, :], 0.0)

            def load_image(g, per_chunk=False):
                xb = xbs[g % NXB]
                nat = xb[0:128, 0 : NBLK * P]
                pstride = nat.ap[0][0]
                # input DMAs (casting f32 -> xdt): t0, batched interior, t8.
                # per_chunk issues one DMA per block so the first chunks land
                # as early as possible (used for the first image only).
                nc.gpsimd.dma_start(out=xb[0:128, 5:1029], in_=x_d[g, 0:128, :])
                dimg = x_d[g]
                groups = ((1, 1), (2, 1), (3, 1), (4, 1), (5, 1), (6, 1), (7, 1)) \
                    if per_chunk else ((1, 4), (5, 3))
                for lo, n in groups:
                    dsrc = AP(dimg.tensor, dimg.offset + (OB * lo - R) * W,
                              [[W, 128], [OB * W, n], [1, W]])
                    ddst = AP(nat.tensor, nat.offset + lo * P + 5,
                              [[pstride, 128], [P, n], [1, W]])
                    nc.gpsimd.dma_start(out=ddst, in_=dsrc)
                nc.gpsimd.dma_start(
                    out=xb[0:68, 8 * P + 5 : 8 * P + 1029], in_=x_d[g, 956:1024, :]
                )

            def image(g):
                xb = xbs[g % NXB]
                ob = obs[g % NOB]

                grouped = False  # 3-chunk group scans measured slower (76.8us vs 72.5us)
                cg = None
                order = list(range(NBLK))
                for t in order:
                    os_, orows, is_, irows, wk = BLOCKS[t]
                    if grouped:
                        if t % 3 == 0:
                            cg = c_pool.tile([128, GW], f16, tag="cg")
                            nc.vector.tensor_tensor_scan(
                                out=cg[0:128, :],
                                data0=zeros[0:128, :],  # unused (grouped=False)
                                data1=xb[0:128, P * t : P * t + GW],
                                initial=0.0,
                                op0=add,
                                op1=add,
                            )
                        off = (t % 3) * P
                        c = cg[0:128, off : off + CW]
                    else:
                        ct = cs_pool.tile([128, CW], f16, tag="c")
                        nc.vector.tensor_tensor_scan(
                            out=ct[0:irows, :],
                            data0=zeros[0:irows, 0:CW],
                            data1=xb[0:irows, P * t : P * t + CW],
                            initial=0.0,
                            op0=add,
                            op1=add,
                        )
                        c = ct
                    ps = ps_pool.tile([128, 1024], f32, tag="ps")
                    wp = w_sb[0:irows, 128 * wk : 128 * wk + orows]
                    wm = w_sb[0:irows, 128 * (wk + 3) : 128 * (wk + 3) + orows]
                    nc.tensor.matmul(ps[0:orows, 0:512], wp, c[0:irows, 9:521],
                                     start=True, stop=False)
                    nc.tensor.matmul(ps[0:orows, 0:512], wm, c[0:irows, 0:512],
                                     start=False, stop=True)
                    nc.tensor.matmul(ps[0:orows, 512:1024], wp, c[0:irows, 521:1033],
                                     start=True, stop=False)
                    nc.tensor.matmul(ps[0:orows, 512:1024], wm, c[0:irows, 512:1024],
                                     start=False, stop=True)
                    dst = ob[0:orows, 1024 * t : 1024 * t + 1024]
                    if t < DVE_DRAINS or (g == IPC - 1 and t == 8):
                        nc.vector.tensor_copy(out=dst, in_=ps[0:orows, :])
                    else:
                        nc.scalar.copy(dst, ps[0:orows, :])

                # output DMAs: batched chunk pairs, then t8; the last
                # image's tail is split finer and spread across queues so the
                # final transfers overlap their drains
                dout = o_d[g]
                onat = ob[0:128, 0 : NBLK * 1024]
                opstride = onat.ap[0][0]
                last = g == IPC - 1
                parts = ((0, 2), (2, 2), (4, 2), (6, 1), (7, 1)) if last \
                    else ((0, 2), (2, 2), (4, 2), (6, 2))
                for lo, n in parts:
                    ddram = AP(dout.tensor, dout.offset + OB * lo * W,
                               [[W, OB], [OB * W, n], [1, W]])
                    osrc = AP(onat.tensor, onat.offset + lo * 1024,
                              [[opstride, OB], [1024, n], [1, 1024]])
                    nc.sync.dma_start(out=ddram, in_=osrc)
                eng_o = nc.scalar if last else nc.sync
                eng_o.dma_start(
                    out=o_d[g, 960:1024, :], in_=ob[0:64, 8 * 1024 : 9 * 1024]
                )

            for xb_ in xbs[1:]:
                pads(xb_, nc.vector)

            for _ in range(reps):
                load_image(0, per_chunk=True)
                load_image(1, per_chunk=True)
                for g in range(IPC):
                    if g + 2 < IPC:
                        load_image(g + 2, per_chunk=True)
                    image(g)

    nc.compile()
    return nc


_LOCK = threading.Lock()
_CACHED = {}


def _get_nc(reps: int = 1):
    with _LOCK:
        key = ("nc", reps)
        if key not in _CACHED:
            _CACHED[key] = _build(reps)
        return _CACHED[key]


def _postprocess(out48_i8: np.ndarray) -> np.ndarray:
    out = out48_i8.astype(np.float32).reshape(B, C, H, W) * (1.0 / OUT_SCALE)
    r = np.arange(H)
    hc = (np.minimum(r + R, W - 1) - np.maximum(r - R, 0) + 1).astype(np.float32)
    out[..., 0:R] *= (9.0 / hc[0:R])[None, None, None, :]
    out[..., W - R : W] *= (9.0 / hc[W - R : W])[None, None, None, :]
    return out


def run(x: np.ndarray, trace: bool = False, reps: int = 1):
    from concourse.bass_utils import run_bass_kernel_spmd

    assert x.shape == (B, C, H, W), x.shape
    x48 = np.ascontiguousarray(x.reshape(IMGS, H, W), dtype=np.float32)
    wts = _weights()
    in_maps = [
        {
            "x": np.ascontiguousarray(x48[IPC * c : IPC * (c + 1)]),
            "wts": wts,
        }
        for c in range(NCORES)
    ]
    nc = _get_nc(reps)
    res = run_bass_kernel_spmd(nc, in_maps, core_ids=list(range(NCORES)), trace=trace)
    out48 = np.concatenate([r["out"] for r in res.results], axis=0)
    return _postprocess(out48), res


def kernel(x: np.ndarray) -> np.ndarray:
    out, _ = run(x, trace=False)
    return out


# revision 49
# speedup vs baseline: 1.0070x; 1.0070x over previous
"""BoxFilter 9x9 mean, TRN2 x8 — v6: scan + matmul-differencing.

Per 128-row block (9 overlapping blocks per 1024x1024 image, 6 images/core):
  - casting DMA loads x f32->f16 into a persistent 9-chunk SBUF tile
    (chunks padded with 5 left + 4 right zero cols, zeroed once at startup);
    interior chunks are fetched with one multi-window DMA (overlapping DRAM
    windows, partition-first SBUF AP) to amortize SWDGE descriptor-gen cost
  - one DVE tensor_tensor_scan per chunk produces the horizontal prefix sum
    c (f16 out, fp32 internal state): h[j] = c[j+9] - c[j] is the 9-wide
    horizontally clamped window sum
  - 4 matmuls per block compute the vertical 9-band sum of h directly from c:
    ps = (+W)@c[:, 9:] + (-W)@c[:, :1024], with the 1/(9*vcount) row scale
    folded into +/-W (f16 weights, 3 block kinds x 2 signs)
  - one 1024-wide scalar-engine copy drains the 2-bank PSUM tile to SBUF int8
    (the tolerance is relative to max|out| ~ 0.97, so a uniform int8 grid at
    OUT_SCALE=120 costs only ~4e-3 absolute error); host dequantizes by
    1/OUT_SCALE and applies the edge-column 9/hcount fix
  - chunk-pair batched DMAs write the int8 output back (halves output DMA
    traffic vs f16)

Engine busy per core (TimelineSim): DVE ~63us (scans; 1x rate, no 2x mode
for TensorScalarPtr, and the Pool engine cannot run it), DMA ~55us (f16 in
13.4MB + int8 out 6.3MB at 360GB/s), Act ~55us (drains), PE ~46us.
fp8 input would cut DMA by another 18.5us but its quantization alone
measures 2.9e-2 max-rel error (> 2e-2 budget), so the input stays f16.
"""

import threading

import numpy as np

NCORES = 8
B, C, H, W = 16, 3, 1024, 1024
IMGS = B * C
IPC = IMGS // NCORES
R = 4
OB = 120
NBLK = 9
P = 1040  # SBUF chunk pitch (elements)
CW = 1033  # scan width: 5 left zeros + 1024 + 4 right zeros
GW = 2 * P + CW  # 3-chunk group scan width (chunk gaps are all zeros)

# (out_start, out_rows, in_start, in_rows, weight_kind)
BLOCKS = [(0, OB, 0, 128, 0)]
BLOCKS += [(OB * t, OB, OB * t - R, 128, 1) for t in range(1, 8)]
BLOCKS += [(960, 64, 956, 68, 2)]

INPUT_FP8 = False  # False -> f16 input path
OUT_SCALE = 120.0  # output quantization scale for int8 DRAM writes
POOL_SCANS = 0  # gpsimd cannot run TensorScalarPtr (HW ISA check) — keep 0
DVE_DRAINS = 0  # drains per image on DVE instead of Act


def _weights():
    k = np.arange(128)[:, None].astype(np.int64)
    m = np.arange(128)[None, :].astype(np.int64)
    vc0 = np.minimum(m + R, 1023) - np.maximum(m - R, 0) + 1
    w0 = ((np.maximum(m - R, 0) <= k) & (k <= m + R) & (m < OB)) / (9.0 * vc0)
    wi = ((m <= k) & (k <= m + 2 * R) & (m < OB)) / 81.0
    vc8 = np.maximum(np.minimum(964 + m, 1023) - (956 + m) + 1, 1)
    w8 = ((m <= k) & (k <= np.minimum(m + 2 * R, 67)) & (m < 64)) / (9.0 * vc8)
    wts = np.stack([w0, wi, w8]) * OUT_SCALE
    wts6 = np.concatenate([wts, -wts]).astype(np.float16)  # [6,128,128]
    return np.ascontiguousarray(wts6.transpose(1, 0, 2).reshape(128, 6 * 128))


def _build(reps: int = 1):
    import concourse.bacc as bacc
    import concourse.mybir as mybir
    import concourse.tile as tile
    from concourse.ap import AP

    f32 = mybir.dt.float32
    f16 = mybir.dt.float16
    fp8 = mybir.dt.float8e4
    xdt = fp8 if INPUT_FP8 else f16
    add = mybir.AluOpType.add

    nc = bacc.Bacc("TRN2", target_bir_lowering=False, debug=False, num_devices=NCORES)
    x_d = nc.declare_dram_parameter("x", [IPC, H, W], f32, isOutput=False)
    wts_d = nc.declare_dram_parameter("wts", [128, 6 * 128], f16, isOutput=False)
    i8 = mybir.dt.int8
    o_d = nc.declare_dram_parameter("out", [IPC, H, W], i8, isOutput=True)

    NXB = 3
    NOB = 4
    with tile.TileContext(nc) as tc:
        with (
            tc.tile_pool(name="consts", bufs=1) as cpool,
            tc.tile_pool(name="xb", bufs=NXB) as xb_pool,
            tc.tile_pool(name="cs", bufs=5) as cs_pool,
            tc.tile_pool(name="cg", bufs=3) as c_pool,
            tc.tile_pool(name="ob", bufs=NOB) as ob_pool,
            tc.tile_pool(name="ps", bufs=4, space="PSUM") as ps_pool,
        ):
            w_sb = cpool.tile([128, 6 * 128], f16)
            nc.sync.dma_start(out=w_sb[:, 0:384], in_=wts_d[:, 0:384])
            nc.sync.dma_start(out=w_sb[:, 384:768], in_=wts_d[:, 384:768])
            zeros = cpool.tile([128, CW], f16)

            # persistent multi-buffered input / output tiles
            xbs = [xb_pool.tile([128, NBLK * P], xdt, tag="xb", name=f"xb{i}")
                   for i in range(NXB)]
            obs = [ob_pool.tile([128, NBLK * 1024], i8, tag="ob", name=f"ob{i}")
                   for i in range(NOB)]
            def pads(xb, eng):
                nat = xb[0:128, 0 : NBLK * P]
                pstride = nat.ap[0][0]
                padl = AP(nat.tensor, nat.offset, [[pstride, 128], [P, NBLK], [1, 5]])
                padr = AP(nat.tensor, nat.offset + 1029,
                          [[pstride, 128], [P, NBLK], [1, 11]])
                eng.memset(padl, 0.0)
                eng.memset(padr, 0.0)

            pads(xbs[0], nc.vector)
            nc.vector.memset(zeros[:, :], 0.0)

            def load_image(g, per_chunk=False):
                xb = xbs[g % NXB]
                nat = xb[0:128, 0 : NBLK * P]
                pstride = nat.ap[0][0]
                # input DMAs (casting f32 -> xdt): t0, batched interior, t8.
                # per_chunk issues one DMA per block so the first chunks land
                # as early as possible (used for the first image only).
                nc.gpsimd.dma_start(out=xb[0:128, 5:1029], in_=x_d[g, 0:128, :])
                dimg = x_d[g]
                groups = ((1, 1), (2, 1), (3, 1), (4, 1), (5, 1), (6, 1), (7, 1)) \
                    if per_chunk else ((1, 4), (5, 3))
                for lo, n in groups:
                    dsrc = AP(dimg.tensor, dimg.offset + (OB * lo - R) * W,
                              [[W, 128], [OB * W, n], [1, W]])
                    ddst = AP(nat.tensor, nat.offset + lo * P + 5,
                              [[pstride, 128], [P, n], [1, W]])
                    nc.gpsimd.dma_start(out=ddst, in_=dsrc)
                nc.gpsimd.dma_start(
                    out=xb[0:68, 8 * P + 5 : 8 * P + 1029], in_=x_d[g, 956:1024, :]
                )

            def image(g):
                xb = xbs[g % NXB]
                ob = obs[g % NOB]

                grouped = False  # 3-chunk group scans measured slower (76.8us vs 72.5us)
                cg = None
                order = list(range(NBLK))
                for t in order:
                    os_, orows, is_, irows, wk = BLOCKS[t]
                    if grouped:
                        if t % 3 == 0:
                            cg = c_pool.tile([128, GW], f16, tag="cg")
                            nc.vector.tensor_tensor_scan(
                                out=cg[0:128, :],
                                data0=zeros[0:128, :],  # unused (grouped=False)
                                data1=xb[0:128, P * t : P * t + GW],
                                initial=0.0,
                                op0=add,
                                op1=add,
                            )
                        off = (t % 3) * P
                        c = cg[0:128, off : off + CW]
                    else:
                        ct = cs_pool.tile([128, CW], f16, tag="c")
                        nc.vector.tensor_tensor_scan(
                            out=ct[0:irows, :],
                            data0=zeros[0:irows, 0:CW],
                            data1=xb[0:irows, P * t : P * t + CW],
                            initial=0.0,
                            op0=add,
                            op1=add,
                        )
                        c = ct
                    ps = ps_pool.tile([128, 1024], f32, tag="ps")
                    wp = w_sb[0:irows, 128 * wk : 128 * wk + orows]
                    wm = w_sb[0:irows, 128 * (wk + 3) : 128 * (wk + 3) + orows]
                    nc.tensor.matmul(ps[0:orows, 0:512], wp, c[0:irows, 9:521],
                                     start=True, stop=False)
                    nc.tensor.matmul(ps[0:orows, 0:512], wm, c[0:irows, 0:512],
                                     start=False, stop=True)
                    nc.tensor.matmul(ps[0:orows, 512:1024], wp, c[0:irows, 521:1033],
                                     start=True, stop=False)
                    nc.tensor.matmul(ps[0:orows, 512:1024], wm, c[0:irows, 512:1024],
                                     start=False, stop=True)
                    dst = ob[0:orows, 1024 * t : 1024 * t + 1024]
                    if t < DVE_DRAINS or (g == IPC - 1 and t == 8):
                        nc.vector.tensor_copy(out=dst, in_=ps[0:orows, :])
                    else:
                        nc.scalar.copy(dst, ps[0:orows, :])

                # output DMAs: batched chunk pairs, then t8
                dout = o_d[g]
                onat = ob[0:128, 0 : NBLK * 1024]
                opstride = onat.ap[0][0]
                for lo, n in ((0, 2), (2, 2), (4, 2), (6, 2)):
                    ddram = AP(dout.tensor, dout.offset + OB * lo * W,
                               [[W, OB], [OB * W, n], [1, W]])
                    osrc = AP(onat.tensor, onat.offset + lo * 1024,
                              [[opstride, OB], [1024, n], [1, 1024]])
                    nc.sync.dma_start(out=ddram, in_=osrc)
                nc.sync.dma_start(
                    out=o_d[g, 960:1024, :], in_=ob[0:64, 8 * 1024 : 9 * 1024]
                )

            for xb_ in xbs[1:]:
                pads(xb_, nc.vector)

            for _ in range(reps):
                load_image(0, per_chunk=True)
                load_image(1, per_chunk=True)
                for g in range(IPC):
                    if g + 2 < IPC:
                        load_image(g + 2, per_chunk=True)
                    image(g)

    nc.compile()
    return nc


_LOCK = threading.Lock()
_CACHED = {}


def _get_nc(reps: int = 1):
    with _LOCK:
        key = ("nc", reps)
        if key not in _CACHED:
            _CACHED[key] = _build(reps)
        return _CACHED[key]


def _postprocess(out48_i8: np.ndarray) -> np.ndarray:
    out = out48_i8.astype(np.float32).reshape(B, C, H, W) * (1.0 / OUT_SCALE)
    r = np.arange(H)
    hc = (np.minimum(r + R, W - 1) - np.maximum(r - R, 0) + 1).astype(np.float32)
    out[..., 0:R] *= (9.0 / hc[0:R])[None, None, None, :]
    out[..., W - R : W] *= (9.0 / hc[W - R : W])[None, None, None, :]
    return out


def run(x: np.ndarray, trace: bool = False, reps: int = 1):
    from concourse.bass_utils import run_bass_kernel_spmd

    assert x.shape == (B, C, H, W), x.shape
    x48 = np.ascontiguousarray(x.reshape(IMGS, H, W), dtype=np.float32)
    wts = _weights()
    in_maps = [
        {
            "x": np.ascontiguousarray(x48[IPC * c : IPC * (c + 1)]),
            "wts": wts,
        }
        for c in range(NCORES)
    ]
    nc = _get_nc(reps)
    res = run_bass_kernel_spmd(nc, in_maps, core_ids=list(range(NCORES)), trace=trace)
    out48 = np.concatenate([r["out"] for r in res.results], axis=0)
    return _postprocess(out48), res


def kernel(x: np.ndarray) -> np.ndarray:
    out, _ = run(x, trace=False)
    return out
